# revision 14
# baseline (speedup 1.0000x reference)
"""Trainium2 Bass kernel for nn_BayesPosLinear (S=16, B=32, IN=OUT=1024).

  std_w = 1e-6 + softplus(w_std_eta)            [OUT, IN]
  w     = exp(w_mu + std_w * eps_w)             [B, OUT, IN]
  b     = b_mu + (1e-6 + softplus(b_std_eta)) * eps_b   [B, OUT]
  out   = einsum('sbi,boi->sbo', input, w) + b  [S, B, OUT]
  kl_w  = sum(-log std_w + 0.5(std_w^2 + w_mu^2) - 0.5)
  kl_b  = likewise over (b_mu, std_b)

Sharding: OUT split 8 ways (128 rows per core). Per core this minimizes
HBM traffic (16 MiB eps shard + 1 MiB replicated params + 2 MiB input)
versus batch sharding (which would replicate the 8 MiB w_mu/w_std_eta).
Each core computes its output column block plus partial KL sums; the
host concatenates blocks and adds the partials. No collectives needed.

Device layout (per core, o = this core's 128 OUT rows, i = ih*128 + p):
  epsT [b, p, ih*128+o]   bf16  - eps_w transposed so the contraction
                                  index i sits on SBUF partitions
  inT  [p, b*128+ih*16+s] bf16  - input as matmul stationary chunks
  wmuT/wetaT [p, ih*128+o] f32
Main loop per group of G batches: DVE mul (std*eps, bf16 2x mode),
ACT exp, DVE mul (exp(w_mu)*u), then per batch 8 accumulating K=128
matmuls (stationary=input chunk [128,16], moving=w chunk [128,128]) on
top of a K=1 bias matmul pair (bias split hi+lo bf16 for f32 accuracy).
"""

import os
import numpy as np
import concourse.bass as bass
import concourse.tile as tile
from concourse import mybir
from concourse.bass_utils import run_bass_kernel_spmd
from concourse.vector_clock import ScopedClock

AF = mybir.ActivationFunctionType
F32 = mybir.dt.float32
HALF = mybir.dt.float16  # fp16: 10 mantissa bits, same speed as bf16 on PE/DVE
ALU = mybir.AluOpType

S, B, IN, OUT = 16, 32, 1024, 1024
N_CORES = 8
P = 128                 # SBUF partitions = i_lo chunk
IH = IN // P            # 8 contraction chunks
O = OUT // N_CORES      # 128 out rows per core
G = 4                   # batches per group
NG = B // G
FD = G * IH * O         # free dim of a group tile (4096)

_HALF_NP = np.dtype(np.float16)

# ---------------------------------------------------------------------------
# Workaround for this container's walrus build: CoreV3GenImpl::setupSyncWait
# rejects instructions carrying more than _MAX_WAITS semaphore waits ("Too
# many sync wait commands"), but Tile freely emits 3+ (and puts one wait per
# outstanding proc on the tail drain). Post-pass: move excess semaphore
# waits onto no-op instructions inserted just before, on the same engine.
# ---------------------------------------------------------------------------
_MAX_WAITS = 1


def _split_excess_waits(nc: bass.Bass, max_waits: int = _MAX_WAITS) -> None:
    for fn in nc.m.functions:
        for blk in fn.blocks:
            out, changed = [], False
            for inst in blk.instructions:
                si = inst.sync_info
                if si is not None and si.on_wait and len(si.on_wait) > max_waits:
                    waits = list(si.on_wait)
                    sem_waits = [w for w in waits if w.sync_type == "semaphore"]
                    other = [w for w in waits if w.sync_type != "semaphore"]
                    keep_n = max(0, max_waits - len(other))
                    keep, rest = sem_waits[:keep_n], sem_waits[keep_n:]
                    for i in range(0, len(rest), max_waits):
                        nop = mybir.InstNoOp(
                            name=f"{inst.name}-wsplit{i}",
                            engine=inst.engine,
                            bass_nofuse=True,
                            sync_info=mybir.SyncInfo(
                                on_wait=rest[i : i + max_waits], on_update=[]
                            ),
                        )
                        out.append(nop)
                    inst.sync_info = mybir.SyncInfo(
                        on_wait=other + keep, on_update=list(si.on_update or [])
                    )
                    changed = True
                out.append(inst)
            if changed:
                blk.instructions = out


# ---------------------------------------------------------------------------
# Device program
# ---------------------------------------------------------------------------
def build_nc(repeats: int = 1, loop_trips: int | None = None) -> bass.Bass:
    """Build the per-core Bass program. `repeats` unrolls the whole body
    multiple times; `loop_trips` additionally wraps the unrolled body in a
    hardware For_i loop. Both are used only for wall-clock slope timing
    (the axon client exposes no NTFF profiling, and per-dispatch overhead
    is ~3 ms, so the body must repeat enough to dominate it)."""
    nc = bass.Bass(
        "TRN2",
        target_bir_lowering=False,
        debug=False,
        enable_asserts=True,
        num_devices=1,
    )
    epsT = nc.dram_tensor("epsT", [B, P, IH * O], HALF, kind="ExternalInput").ap()
    inT = nc.dram_tensor("inT", [P, B * IH * S], HALF, kind="ExternalInput").ap()
    wmuT = nc.dram_tensor("wmuT", [P, IH * O], F32, kind="ExternalInput").ap()
    wetaT = nc.dram_tensor("wetaT", [P, IH * O], F32, kind="ExternalInput").ap()
    bmu_rep = nc.dram_tensor("bmu_rep", [B, O], F32, kind="ExternalInput").ap()
    beta_rep = nc.dram_tensor("beta_rep", [B, O], F32, kind="ExternalInput").ap()
    epsb = nc.dram_tensor("epsb", [B, O], F32, kind="ExternalInput").ap()
    out_d = nc.dram_tensor("out", [S, B, O], F32, kind="ExternalOutput").ap()
    kl_d = nc.dram_tensor("kl", [1, 2], F32, kind="ExternalOutput").ap()

    with tile.TileContext(nc) as tc:
        with (
            tc.tile_pool(name="const", bufs=1) as const,
            tc.tile_pool(name="scratch", bufs=2) as scratch,
            tc.tile_pool(name="eps", bufs=4) as eps_pool,
            tc.tile_pool(name="t", bufs=3) as t_pool,
            tc.tile_pool(name="u", bufs=3) as u_pool,
            tc.tile_pool(name="wv", bufs=3) as wv_pool,
            tc.tile_pool(name="osb", bufs=3) as osb_pool,
            tc.tile_pool(name="ps", bufs=4, space="PSUM") as ps_pool,
            tc.tile_pool(name="pskl", bufs=1, space="PSUM") as pskl_pool,
        ):
            def emit_body():
                ones_bf = const.tile([1, S], HALF, tag="ones_bf")
                nc.vector.memset(ones_bf[:], 1.0)
                ones_f = const.tile([P, 1], F32, tag="ones_f")
                nc.vector.memset(ones_f[:], 1.0)

                # ---- persistent loads -------------------------------------
                in_t = const.tile([P, B * IH * S], HALF, tag="in_t")
                nc.sync.dma_start(in_t[:], inT[:])
                wmu_t = const.tile([P, IH * O], F32, tag="wmu_t")
                nc.sync.dma_start(wmu_t[:], wmuT[:])
                weta_t = const.tile([P, IH * O], F32, tag="weta_t")
                nc.sync.dma_start(weta_t[:], wetaT[:])
                bmu_t = const.tile([B, O], F32, tag="bmu_t")
                nc.sync.dma_start(bmu_t[:], bmu_rep[:])
                beta_t = const.tile([B, O], F32, tag="beta_t")
                nc.sync.dma_start(beta_t[:], beta_rep[:])
                epsb_t = const.tile([B, O], F32, tag="epsb_t")
                nc.sync.dma_start(epsb_t[:], epsb[:])

                # ---- std_w / E = exp(w_mu), replicated G times ------------
                # softplus(x) = ln(exp(x) + 1); no Softplus LUT in this build
                ex_t = scratch.tile([P, IH * O], F32, tag="sp_exp")
                nc.scalar.activation(ex_t[:], weta_t[:], AF.Exp)
                sp_t = scratch.tile([P, IH * O], F32, tag="sp")
                nc.scalar.activation(sp_t[:], ex_t[:], AF.Ln, bias=1.0)
                std_f = const.tile([P, IH * O], F32, tag="std_f")
                nc.vector.tensor_scalar_add(std_f[:], sp_t[:], 1e-6)
                std_bf = const.tile([P, IH * O], HALF, tag="std_bf")
                nc.vector.tensor_copy(std_bf[:], std_f[:])
                e_bf = const.tile([P, IH * O], HALF, tag="e_bf")
                nc.scalar.activation(e_bf[:], wmu_t[:], AF.Exp)
                std_rep = const.tile([P, FD], HALF, tag="std_rep")
                e_rep = const.tile([P, FD], HALF, tag="e_rep")
                std_rep_v = std_rep[:].rearrange("p (g f) -> p g f", g=G)
                e_rep_v = e_rep[:].rearrange("p (g f) -> p g f", g=G)
                for g in range(G):
                    nc.vector.tensor_copy(std_rep_v[:, g, :], std_bf[:])
                    nc.vector.tensor_copy(e_rep_v[:, g, :], e_bf[:])

                # ---- KL(w) pieces -----------------------------------------
                junk = scratch.tile([P, IH * O], F32, tag="junk")
                ln_acc = scratch.tile([P, 1], F32, tag="ln_acc")
                s2_acc = scratch.tile([P, 1], F32, tag="s2_acc")
                m2_acc = scratch.tile([P, 1], F32, tag="m2_acc")
                nc.scalar.activation(junk[:], std_f[:], AF.Ln, accum_out=ln_acc[:])
                nc.scalar.activation(junk[:], std_f[:], AF.Square, accum_out=s2_acc[:])
                nc.scalar.activation(junk[:], wmu_t[:], AF.Square, accum_out=m2_acc[:])
                sm_acc = scratch.tile([P, 1], F32, tag="sm_acc")
                nc.vector.tensor_add(sm_acc[:], s2_acc[:], m2_acc[:])
                klw_vec = scratch.tile([P, 1], F32, tag="klw_vec")
                nc.vector.scalar_tensor_tensor(
                    klw_vec[:], sm_acc[:], 0.5, ln_acc[:], ALU.mult, ALU.subtract
                )
                ps_kl = pskl_pool.tile([1, 1], F32, tag="ps_kl")
                nc.tensor.matmul(ps_kl[:], ones_f[:], klw_vec[:], start=True, stop=True)

                # ---- bias vector + KL(b) ----------------------------------
                exb = scratch.tile([B, O], F32, tag="exb")
                nc.scalar.activation(exb[:], beta_t[:], AF.Exp)
                spb = scratch.tile([B, O], F32, tag="spb")
                nc.scalar.activation(spb[:], exb[:], AF.Ln, bias=1.0)
                stdb = scratch.tile([B, O], F32, tag="stdb")
                nc.vector.tensor_scalar_add(stdb[:], spb[:], 1e-6)
                prodb = scratch.tile([B, O], F32, tag="prodb")
                nc.vector.tensor_mul(prodb[:], stdb[:], epsb_t[:])
                bv32 = scratch.tile([B, O], F32, tag="bv32")
                nc.vector.tensor_add(bv32[:], prodb[:], bmu_t[:])
                bv_hi = scratch.tile([B, O], HALF, tag="bv_hi")
                nc.vector.tensor_copy(bv_hi[:], bv32[:])
                bv_lo = scratch.tile([B, O], HALF, tag="bv_lo")
                nc.vector.tensor_sub(bv_lo[:], bv32[:], bv_hi[:])
                bvrow_hi = const.tile([1, B * O], HALF, tag="bvrow_hi")
                bvrow_lo = const.tile([1, B * O], HALF, tag="bvrow_lo")
                nc.sync.dma_start(bvrow_hi[:], bv_hi[:])
                nc.sync.dma_start(bvrow_lo[:], bv_lo[:])

                lnb = scratch.tile([1, 1], F32, tag="lnb")
                s2b = scratch.tile([1, 1], F32, tag="s2b")
                m2b = scratch.tile([1, 1], F32, tag="m2b")
                junkb = scratch.tile([1, O], F32, tag="junkb")
                nc.scalar.activation(junkb[:], stdb[0:1, :], AF.Ln, accum_out=lnb[:])
                nc.scalar.activation(junkb[:], stdb[0:1, :], AF.Square, accum_out=s2b[:])
                nc.scalar.activation(junkb[:], bmu_t[0:1, :], AF.Square, accum_out=m2b[:])
                smb = scratch.tile([1, 1], F32, tag="smb")
                nc.vector.tensor_add(smb[:], s2b[:], m2b[:])
                klb_val = scratch.tile([1, 1], F32, tag="klb_val")
                nc.vector.scalar_tensor_tensor(
                    klb_val[:], smb[:], 0.5, lnb[:], ALU.mult, ALU.subtract
                )

                kl_sb = scratch.tile([1, 2], F32, tag="kl_sb")
                nc.vector.tensor_scalar_add(
                    kl_sb[0:1, 0:1], ps_kl[:], -0.5 * float(P * IH * O)
                )
                nc.vector.tensor_scalar_add(
                    kl_sb[0:1, 1:2], klb_val[:], -0.5 * float(O)
                )
                nc.sync.dma_start(kl_d[:], kl_sb[:])

                # ---- main loop over batch groups --------------------------
                in_v = in_t[:].rearrange("p (b ih s) -> p b ih s", b=B, ih=IH)
                for g in range(NG):
                    eps_t = eps_pool.tile([P, FD], HALF, tag="eps_t")
                    nc.sync.dma_start(
                        eps_t[:].rearrange("p (g f) -> p g f", g=G),
                        epsT[g * G : (g + 1) * G, :, :].rearrange("g p f -> p g f"),
                    )
                    t_t = t_pool.tile([P, FD], HALF, tag="t_t")
                    nc.vector.tensor_mul(t_t[:], eps_t[:], std_rep[:])
                    u_t = u_pool.tile([P, FD], HALF, tag="u_t")
                    nc.scalar.activation(u_t[:], t_t[:], AF.Exp)
                    wv_t = wv_pool.tile([P, FD], HALF, tag="wv_t")
                    nc.vector.tensor_mul(wv_t[:], u_t[:], e_rep[:])

                    ps = ps_pool.tile([S, G * O], F32, tag="ps")
                    bsl = slice(g * G * O, (g + 1) * G * O)
                    nc.tensor.matmul(
                        ps[:], ones_bf[:], bvrow_hi[0:1, bsl], start=True, stop=False
                    )
                    nc.tensor.matmul(
                        ps[:], ones_bf[:], bvrow_lo[0:1, bsl], start=False, stop=False
                    )
                    ps_v = ps[:].rearrange("s (g o) -> s g o", g=G)
                    wv_v = wv_t[:].rearrange("p (g ih o) -> p g ih o", g=G, ih=IH)
                    for g2 in range(G):
                        for ih in range(IH):
                            nc.tensor.matmul(
                                ps_v[:, g2, :],
                                in_v[:, g * G + g2, ih, :],
                                wv_v[:, g2, ih, :],
                                start=False,
                                stop=(g2 == G - 1 and ih == IH - 1),
                            )

                    out_sb = osb_pool.tile([S, G * O], F32, tag="out_sb")
                    if g % 2 == 0:
                        nc.scalar.copy(out_sb[:], ps[:])
                    else:
                        nc.vector.tensor_copy(out_sb[:], ps[:])
                    nc.sync.dma_start(
                        out_d[:, g * G : (g + 1) * G, :],
                        out_sb[:].rearrange("s (g o) -> s g o", g=G),
                    )

            if loop_trips is None:
                for _ in range(repeats):
                    emit_body()
            else:
                ET = mybir.EngineType
                with tc.For_i(
                    0,
                    loop_trips,
                    1,
                    hint_engines=(ET.PE, ET.DVE, ET.Activation, ET.SP, ET.Pool),
                ):
                    for _ in range(repeats):
                        emit_body()
    _split_excess_waits(nc)
    return nc


# ---------------------------------------------------------------------------
# Host side: shard / layout prep, run, gather
# ---------------------------------------------------------------------------
def prep_in_maps(input, w_mu, w_std_eta, b_mu, b_std_eta, eps_w, eps_b):
    input = np.asarray(input, dtype=np.float32)
    w_mu = np.asarray(w_mu, dtype=np.float32)
    w_std_eta = np.asarray(w_std_eta, dtype=np.float32)
    b_mu = np.asarray(b_mu, dtype=np.float32)
    b_std_eta = np.asarray(b_std_eta, dtype=np.float32)
    eps_w = np.asarray(eps_w, dtype=np.float32)
    eps_b = np.asarray(eps_b, dtype=np.float32)

    # input [S, B, IN] -> inT [p, (b, ih, s)] bf16 (shared by all cores)
    inT = np.ascontiguousarray(
        input.reshape(S, B, IH, P).transpose(3, 1, 2, 0)
    ).reshape(P, B * IH * S).astype(_HALF_NP)

    in_maps = []
    for c in range(N_CORES):
        osl = slice(c * O, (c + 1) * O)
        # eps_w [B, OUT, IN] -> [b, p, (ih, o)] bf16
        epsT = np.ascontiguousarray(
            eps_w[:, osl, :].reshape(B, O, IH, P).transpose(0, 3, 2, 1)
        ).reshape(B, P, IH * O).astype(_HALF_NP)
        wmuT = np.ascontiguousarray(
            w_mu[osl, :].reshape(O, IH, P).transpose(2, 1, 0)
        ).reshape(P, IH * O)
        wetaT = np.ascontiguousarray(
            w_std_eta[osl, :].reshape(O, IH, P).transpose(2, 1, 0)
        ).reshape(P, IH * O)
        in_maps.append(
            {
                "epsT": epsT,
                "inT": inT,
                "wmuT": wmuT,
                "wetaT": wetaT,
                "bmu_rep": np.ascontiguousarray(
                    np.broadcast_to(b_mu[osl], (B, O))
                ),
                "beta_rep": np.ascontiguousarray(
                    np.broadcast_to(b_std_eta[osl], (B, O))
                ),
                "epsb": np.ascontiguousarray(eps_b[:, osl]),
            }
        )
    return in_maps


def gather_outputs(results):
    out = np.concatenate([r["out"] for r in results], axis=2)
    kl_w = np.float32(sum(float(r["kl"][0, 0]) for r in results))
    kl_b = np.float32(sum(float(r["kl"][0, 1]) for r in results))
    return out, np.asarray(kl_w), np.asarray(kl_b)


_NC_CACHE = {}


def get_nc(repeats: int = 1, loop_trips: int | None = None) -> bass.Bass:
    key = (repeats, loop_trips)
    if key not in _NC_CACHE:
        _NC_CACHE[key] = build_nc(repeats, loop_trips)
    return _NC_CACHE[key]


def kernel(input, w_mu, w_std_eta, b_mu, b_std_eta, eps_w, eps_b):
    in_maps = prep_in_maps(input, w_mu, w_std_eta, b_mu, b_std_eta, eps_w, eps_b)
    nc = get_nc()
    for _ in range(3):
        res = run_bass_kernel_spmd(nc, in_maps, core_ids=list(range(N_CORES)))
        out, kl_w, kl_b = gather_outputs(res.results)
        if np.isfinite(out).all() and np.isfinite(kl_w) and np.isfinite(kl_b):
            break
    return out, kl_w, kl_b
